# revision 1
# baseline (speedup 1.0000x reference)
"""Causal self-attention (B=4, T=2048, C=1024, H=16, D=64) on 8 TRN2 cores.

Sharding: core c handles batch b = c//2 and head-half hh = c%2 (8 heads).
Each core computes qkv for its heads, attention, and a partial output
projection; the host sums the two partials per batch and adds b_proj.

Device kernel:
  - x transposed on PE (xT: [cin, t]) so qkv matmuls contract over cin.
  - q,k produced transposed per head-pair: qT/kT [128, T] bf16, partitions
    0:64 = head 2p, 64:128 = head 2p+1 (PE row-tiling runs the two K=64
    score matmuls concurrently).
  - scores as S^T [k, q] (k on partitions); att@v as out^T = v.T @ expS^T;
    v carries a ones column so the same accumulation produces the softmax
    denominator in output partition 64.
  - softmax normalization: DVE reciprocal + GPSIMD partition_broadcast +
    one DVE multiply per head.
  - fp32r (full-rate fp32) for qkv/proj matmuls; bf16 attention operands.
  - emission interleaves qkv chunk t4+1 into attention chunk qc=t4 at
    work-unit granularity so PE fills ACT-bound softmax stalls.
"""

import os

import numpy as np

import concourse.mybir as mybir
import concourse.tile as tile
from concourse import bacc
from concourse.bass_utils import run_bass_kernel_spmd

B, T, C = 4, 2048, 1024
H, D = 16, 64
HH = 512  # per-core head width: 8 heads * 64
N_CORES = 8

f32 = mybir.dt.float32
f32r = mybir.dt.float32r
bf16 = mybir.dt.bfloat16
EXP = mybir.ActivationFunctionType.Exp

_BUILT = None
LAST_RESULT = None  # BassKernelResults of the most recent run (for profiling)


def _interleave(a, b):
    """Merge unit lists: spread b evenly through a."""
    out = []
    na, nb = len(a), len(b)
    if na == 0:
        return list(b)
    bi = 0
    for i, u in enumerate(a):
        out.append(u)
        while bi < nb and (bi + 1) * na <= (i + 1) * nb:
            out.append(b[bi])
            bi += 1
    out.extend(b[bi:])
    return out


def _build():
    nc = bacc.Bacc("TRN2", target_bir_lowering=False, debug=False)

    x_d = nc.dram_tensor("xbT", [C, T], f32r, kind="ExternalInput")
    wq_d = nc.dram_tensor("wq", [C, HH], f32r, kind="ExternalInput")
    wk_d = nc.dram_tensor("wk", [C, HH], f32r, kind="ExternalInput")
    wv_d = nc.dram_tensor("wv", [C, HH], f32r, kind="ExternalInput")
    bq_d = nc.dram_tensor("bq", [HH], f32, kind="ExternalInput")
    bk_d = nc.dram_tensor("bk", [HH], f32, kind="ExternalInput")
    bv_d = nc.dram_tensor("bv", [HH], f32, kind="ExternalInput")
    wp_d = nc.dram_tensor("wp", [HH, C], f32r, kind="ExternalInput")
    y_d = nc.dram_tensor("y", [T, C], f32, kind="ExternalOutput")

    with tile.TileContext(nc) as tc:
        with (
            tc.tile_pool(name="persist", bufs=1) as P0,
            tc.tile_pool(name="psum", bufs=3, space="PSUM") as PS,
            tc.tile_pool(name="acc", bufs=1, space="PSUM") as PA,
            tc.tile_pool(name="wpool", bufs=1) as PW,
            tc.tile_pool(name="ph1", bufs=2) as P1,
            tc.tile_pool(name="ph2", bufs=2) as P2,
            tc.tile_pool(name="oTp", bufs=2) as P2o,
            tc.tile_pool(name="expp", bufs=4) as PEx,
        ):
            # Multiplicative causal masks for the 4 diagonal-crossing
            # positions: keep S^T[k, q] iff q - k - 128*d >= 0.
            masks = []
            for d in range(4):
                m = P0.tile([128, 512], bf16, tag=f"mask{d}", name=f"mask{d}")
                nc.gpsimd.memset(m[:, :], 1.0)
                nc.gpsimd.affine_select(
                    out=m[:, :],
                    in_=m[:, :],
                    compare_op=mybir.AluOpType.is_ge,
                    fill=0.0,
                    base=-128 * d,
                    pattern=[[1, 512]],
                    channel_multiplier=-1,
                )
                masks.append(m)

            # ones_row: row 0 = 1.0, rest 0 (bias injection via extra
            # contraction block in the v matmul)
            ones_stage = P0.tile([128, 128], f32, tag="ones_stage")
            nc.gpsimd.memset(ones_stage[:, :], 0.0)
            nc.gpsimd.memset(ones_stage[0:1, :], 1.0)
            ones_row = P0.tile([128, 128], f32r, tag="ones_row")
            nc.vector.tensor_copy(ones_row[:, :], ones_stage[:, :])

            bqk_sb = P0.tile([128, 8], f32, tag="bqk")
            for p in range(4):
                nc.sync.dma_start(
                    bqk_sb[:, p : p + 1], bq_d[128 * p : 128 * (p + 1), None]
                )
                nc.sync.dma_start(
                    bqk_sb[:, 4 + p : 5 + p], bk_d[128 * p : 128 * (p + 1), None]
                )
            bv_stage = P0.tile([128, 512], f32, tag="bv_stage")
            nc.gpsimd.memset(bv_stage[:, :], 0.0)
            nc.sync.dma_start(bv_stage[0:1, :], bv_d[None, :])
            bv_row = P0.tile([128, 512], f32r, tag="bv_row")
            nc.vector.tensor_copy(bv_row[:, :], bv_stage[:, :])

            wp_sb = P0.tile([128, 4, C], f32r, tag="wp")
            nc.sync.dma_start(
                wp_sb[:, :, :], wp_d[:, :].rearrange("(p u) c -> u p c", u=128)
            )

            qT = [
                P0.tile([128, T], bf16, tag=f"qT{p}", name=f"qT{p}")
                for p in range(4)
            ]
            kT = [
                P0.tile([128, T], bf16, tag=f"kT{p}", name=f"kT{p}")
                for p in range(4)
            ]
            # v with a ones column per head: [t, kb, head, 65]; column 64
            # is 1.0 so att@v also accumulates the softmax denominator.
            v_sb = P0.tile([128, 16, 8, 65], bf16, tag="v")
            nc.gpsimd.memset(v_sb[:, :, :, 64:65], 1.0)

            # Resident weights
            wvt = PW.tile([128, 8, HH], f32r, tag="wv")
            nc.sync.dma_start(
                wvt[:, :, :], wv_d[:, :].rearrange("(s u) m -> u s m", u=128)
            )
            wqt, wkt = [], []
            for p in range(4):
                wq_t = PW.tile([128, 8, 128], f32r, tag=f"wq{p}", name=f"wq{p}")
                nc.sync.dma_start(
                    wq_t[:, :, :],
                    wq_d[:, 128 * p : 128 * (p + 1)].rearrange(
                        "(s u) m -> u s m", u=128
                    ),
                )
                wqt.append(wq_t)
                wk_t = PW.tile([128, 8, 128], f32r, tag=f"wk{p}", name=f"wk{p}")
                nc.sync.dma_start(
                    wk_t[:, :, :],
                    wk_d[:, 128 * p : 128 * (p + 1)].rearrange(
                        "(s u) m -> u s m", u=128
                    ),
                )
                wkt.append(wk_t)

            # ---------- work-unit builders ----------

            def qkv_chunk_units(t4):
                """qkv for tokens [t4*512, (t4+1)*512): transposes, v, qT/kT."""
                units = []
                cell = {}

                def u_load(tbl, t4=t4, cell=cell):
                    if "xTc" not in cell:
                        cell["xTc"] = P1.tile(
                            [128, 8, 512], f32r, tag="xT", name="xTc"
                        )
                    xTc = cell["xTc"]
                    tb = 4 * t4 + tbl
                    nc.sync.dma_start(
                        xTc[:, :, tbl * 128 : (tbl + 1) * 128],
                        x_d[:, :].rearrange("(s u) t -> u s t", u=128)[
                            :, :, tb * 128 : (tb + 1) * 128
                        ],
                    )

                def u_v(tbl, t4=t4, cell=cell):
                    xTc = cell["xTc"]
                    tb = 4 * t4 + tbl
                    psv = PS.tile([128, 1024], f32, tag="s", name="psv")
                    for s in range(9):
                        lhsT = (
                            xTc[:, s, tbl * 128 : (tbl + 1) * 128]
                            if s < 8
                            else ones_row[:, :]
                        )
                        rhs = wvt[:, s, :] if s < 8 else bv_row[:, :]
                        nc.tensor.matmul(
                            psv[:, 0:512],
                            lhsT,
                            rhs,
                            start=(s == 0),
                            stop=(s == 8),
                        )
                    nc.vector.tensor_copy(
                        v_sb[:, tb, :, 0:64],
                        psv[:, 0:512].rearrange("p (h d) -> p h d", h=8),
                    )

                def u_qk(p, t4=t4, cell=cell):
                    xTc = cell["xTc"]
                    ps = PS.tile([128, 1024], f32, tag="s", name="psqk")
                    for s in range(8):
                        rhs = xTc[:, s, :]
                        nc.tensor.matmul(
                            ps[:, 0:512],
                            wqt[p][:, s, :],
                            rhs,
                            start=(s == 0),
                            stop=(s == 7),
                        )
                        nc.tensor.matmul(
                            ps[:, 512:1024],
                            wkt[p][:, s, :],
                            rhs,
                            start=(s == 0),
                            stop=(s == 7),
                        )
                    nc.vector.tensor_scalar_add(
                        qT[p][:, t4 * 512 : (t4 + 1) * 512],
                        ps[:, 0:512],
                        bqk_sb[:, p : p + 1],
                    )
                    nc.vector.tensor_scalar_add(
                        kT[p][:, t4 * 512 : (t4 + 1) * 512],
                        ps[:, 512:1024],
                        bqk_sb[:, 4 + p : 5 + p],
                    )

                for tbl in range(4):
                    units.append(lambda tbl=tbl: u_load(tbl))
                    units.append(lambda tbl=tbl: u_v(tbl))
                for p in range(4):
                    units.append(lambda p=p: u_qk(p))
                return units

            def att_chunk_units(qc):
                """Attention + projection for queries [qc*512, (qc+1)*512)."""
                units = []
                cell = {}
                kmax = 4 * qc + 4

                def u_pair_start(p, cell=cell):
                    cell["oA"] = PA.tile([128, 512], f32, tag="poA", name="poA")
                    cell["oB"] = PA.tile([128, 512], f32, tag="poB", name="poB")

                def u_kb(p, kb, qc=qc, cell=cell, kmax=kmax):
                    ps_s = PS.tile([128, 1024], f32, tag="s", name="ps_s")
                    ksl = slice(kb * 128, (kb + 1) * 128)
                    qsl = slice(qc * 512, (qc + 1) * 512)
                    nc.tensor.matmul(
                        ps_s[:, 0:512],
                        kT[p][0:64, ksl],
                        qT[p][0:64, qsl],
                        start=True,
                        stop=True,
                    )
                    nc.tensor.matmul(
                        ps_s[:, 512:1024],
                        kT[p][64:128, ksl],
                        qT[p][64:128, qsl],
                        start=True,
                        stop=True,
                    )
                    e2 = PEx.tile([128, 1024], bf16, tag="e", name="e2")
                    nc.scalar.activation(e2[:, :], ps_s[:, :], EXP, scale=0.125)
                    dg = kb - 4 * qc
                    if dg >= 0:
                        nc.vector.tensor_mul(
                            e2[:, 0:512], e2[:, 0:512], masks[dg][:, :]
                        )
                        nc.vector.tensor_mul(
                            e2[:, 512:1024], e2[:, 512:1024], masks[dg][:, :]
                        )
                    first, last = kb == 0, kb == kmax - 1
                    nc.tensor.matmul(
                        cell["oA"][0:65, :],
                        v_sb[:, kb, 2 * p, :],
                        e2[:, 0:512],
                        start=first,
                        stop=last,
                    )
                    nc.tensor.matmul(
                        cell["oB"][0:65, :],
                        v_sb[:, kb, 2 * p + 1, :],
                        e2[:, 512:1024],
                        start=first,
                        stop=last,
                    )

                def u_norm(p, cell=cell):
                    if "oT" not in cell:
                        cell["oT"] = P2o.tile(
                            [128, 4, 512], f32r, tag="oT", name="oT"
                        )
                    oT = cell["oT"]
                    rcA = P2.tile([1, 512], f32, tag="rcA", name="rcA")
                    rcB = P2.tile([1, 512], f32, tag="rcB", name="rcB")
                    nc.vector.reciprocal(rcA[:, :], cell["oA"][64:65, :])
                    nc.vector.reciprocal(rcB[:, :], cell["oB"][64:65, :])
                    bcA = P2.tile([64, 512], f32, tag="bcA", name="bcA")
                    bcB = P2.tile([64, 512], f32, tag="bcB", name="bcB")
                    nc.gpsimd.partition_broadcast(bcA[:, :], rcA[:, :])
                    nc.gpsimd.partition_broadcast(bcB[:, :], rcB[:, :])
                    nc.vector.tensor_mul(
                        oT[0:64, p, :], cell["oA"][0:64, :], bcA[:, :]
                    )
                    nc.vector.tensor_mul(
                        oT[64:128, p, :], cell["oB"][0:64, :], bcB[:, :]
                    )

                def u_proj(tb, cc, qc=qc, cell=cell):
                    oT = cell["oT"]
                    psy = PS.tile([128, 1024], f32, tag="s", name="psy")
                    for p in range(4):
                        nc.tensor.matmul(
                            psy[:, 0:512],
                            oT[:, p, tb * 128 : (tb + 1) * 128],
                            wp_sb[:, p, cc * 512 : (cc + 1) * 512],
                            start=(p == 0),
                            stop=(p == 3),
                        )
                    yst = P2.tile([128, 512], f32, tag="yst", name="yst")
                    nc.vector.tensor_copy(yst[:, :], psy[:, 0:512])
                    r0 = qc * 512 + tb * 128
                    nc.sync.dma_start(
                        y_d[r0 : r0 + 128, cc * 512 : (cc + 1) * 512],
                        yst[:, :],
                    )

                for p in range(4):
                    units.append(lambda p=p: u_pair_start(p))
                    for kb in range(kmax):
                        units.append(lambda p=p, kb=kb: u_kb(p, kb))
                    units.append(lambda p=p: u_norm(p))
                proj_units = [
                    (lambda tb=tb, cc=cc: u_proj(tb, cc))
                    for tb in range(4)
                    for cc in range(2)
                ]
                return units, proj_units

            # ---------- emission schedule ----------
            # qkv chunk 0 first; then attention(qc) with qkv chunk qc+1
            # spread through it so PE fills ACT-bound softmax stalls.
            for u in qkv_chunk_units(0):
                u()
            pending_proj = []
            for qc in range(4):
                att_units, proj_units = att_chunk_units(qc)
                nxt = pending_proj + (
                    qkv_chunk_units(qc + 1) if qc < 3 else []
                )
                for u in _interleave(att_units, nxt):
                    u()
                pending_proj = proj_units
            for u in pending_proj:
                u()

    nc.finalize()
    return nc


def _get_built():
    global _BUILT
    if _BUILT is None:
        _BUILT = _build()
    return _BUILT


def kernel(**inputs):
    global LAST_RESULT
    x = np.ascontiguousarray(np.asarray(inputs["x"], dtype=np.float32))
    w_qkv = np.ascontiguousarray(np.asarray(inputs["w_qkv"], dtype=np.float32))
    b_qkv = np.ascontiguousarray(np.asarray(inputs["b_qkv"], dtype=np.float32))
    w_proj = np.ascontiguousarray(np.asarray(inputs["w_proj"], dtype=np.float32))
    b_proj = np.ascontiguousarray(np.asarray(inputs["b_proj"], dtype=np.float32))

    nc = _get_built()
    in_maps = []
    for c in range(N_CORES):
        b, hh = c // 2, c % 2
        s = 512 * hh
        in_maps.append(
            {
                "xbT": np.ascontiguousarray(x[b].T),
                "wq": np.ascontiguousarray(w_qkv[:, s : s + 512]),
                "wk": np.ascontiguousarray(w_qkv[:, 1024 + s : 1024 + s + 512]),
                "wv": np.ascontiguousarray(w_qkv[:, 2048 + s : 2048 + s + 512]),
                "bq": np.ascontiguousarray(b_qkv[s : s + 512]),
                "bk": np.ascontiguousarray(b_qkv[1024 + s : 1024 + s + 512]),
                "bv": np.ascontiguousarray(b_qkv[2048 + s : 2048 + s + 512]),
                "wp": np.ascontiguousarray(w_proj[s : s + 512, :]),
            }
        )

    trace = bool(int(os.environ.get("KERNEL_TRACE", "0")))
    res = run_bass_kernel_spmd(
        nc, in_maps, core_ids=list(range(N_CORES)), trace=trace
    )
    LAST_RESULT = res
    out = np.empty((B, T, C), dtype=np.float32)
    for b in range(B):
        out[b] = (
            res.results[2 * b]["y"] + res.results[2 * b + 1]["y"] + b_proj[None, :]
        )
    return out

